# revision 10
# baseline (speedup 1.0000x reference)
"""AdLIF neuron Bass kernel for 8 Trainium2 NeuronCores.

Recurrence per (b, d) element over T timesteps:
    u  = ALPHA_MEM * v + x_t
    s  = (u >= 2 + 0.1 * a)        computed as (0.1*a - u) <= -2  (bit-exact vs ref)
    a' = ALPHA_ADP * a + s
    v' = u - s

Sharding: D (1024) split across 8 cores -> 128 d's per core.
Per core the 32*128 = 4096 (b,d) elements are laid out as
[eh=128 partitions, el=32 free] and time runs in the free dim of a
[128, T*32] SBUF buffer, so each timestep is one [128, 32] slice.
Host pre-reshapes x to [core, eh, t, el] so the per-core DMA is one
fully contiguous 64KB-per-partition stream.
"""

import os
import numpy as np
from contextlib import ExitStack

import concourse.bass as bass
import concourse.tile as tile
from concourse import bacc, mybir
from concourse.bass_utils import run_bass_kernel_spmd

B, T, D = 32, 512, 1024
NCORES = 8
DLOC = D // NCORES          # 128 d's per core
EH, EL = 128, 32            # 4096 elements per core = EH partitions x EL free
NCHUNK = 1                  # t-chunks for DMA pipelining
TC = T // NCHUNK            # 128 timesteps per chunk

ALPHA_MEM = float(np.exp(-1.0 / 20.0))
ALPHA_ADP = float(np.exp(-1.0 / 200.0))

LAST_RESULT = None  # BassKernelResults of the most recent run (for test.py)

F32 = mybir.dt.float32
OP = mybir.AluOpType


def _build():
    nc = bacc.Bacc("TRN2", target_bir_lowering=False, debug=False)
    x_ext = nc.declare_dram_parameter("x", [EH, T * EL], F32, isOutput=False)
    s_ext = nc.declare_dram_parameter("out", [EH, T * EL], F32, isOutput=True)

    with tile.TileContext(nc) as tc, ExitStack() as ctx:
        data = ctx.enter_context(tc.tile_pool(name="data", bufs=1))
        xin = [data.tile([EH, TC * EL], F32, name=f"xin{k}", tag=f"x{k}") for k in range(NCHUNK)]
        sout = [data.tile([EH, TC * EL], F32, name=f"sout{k}", tag=f"s{k}") for k in range(NCHUNK)]

        st = ctx.enter_context(tc.tile_pool(name="state", bufs=1))
        v = st.tile([EH, EL], F32, tag="v")
        a = st.tile([EH, EL], F32, tag="a")
        q = st.tile([EH, EL], F32, tag="q")
        # 1-element scratch: absorbs each chunk's DMA wait in a plain copy
        # (the 3-src STT instruction has a single sync-wait slot, so it must
        # not be the first consumer of a DMA'd tile while also carrying a
        # same-engine dependency wait)
        carrier = st.tile([EH, 1], F32, tag="carrier")

        for k in range(NCHUNK):
            nc.gpsimd.dma_start(xin[k][:], x_ext[:, k * TC * EL:(k + 1) * TC * EL])

        nc.vector.memset(v[:], 0.0)
        nc.vector.memset(a[:], 0.0)

        for t in range(T):
            k, j = divmod(t, TC)
            if j == 0:
                nc.vector.tensor_copy(carrier[:], xin[k][:, 0:1])
            xt = xin[k][:, j * EL:(j + 1) * EL]
            st_ = sout[k][:, j * EL:(j + 1) * EL]
            # u = alpha_mem * v + x_t   (u lives in v)
            nc.vector.scalar_tensor_tensor(v[:], v[:], ALPHA_MEM, xt,
                                           op0=OP.mult, op1=OP.add)
            # q = 0.1 * a - u
            nc.vector.scalar_tensor_tensor(q[:], a[:], 0.1, v[:],
                                           op0=OP.mult, op1=OP.subtract)
            # s = (q <= -2)  -> straight into the output buffer
            nc.vector.tensor_scalar(st_, q[:], -2.0, None, op0=OP.is_le)
            # a = alpha_adp * a + s
            nc.vector.scalar_tensor_tensor(a[:], a[:], ALPHA_ADP, st_,
                                           op0=OP.mult, op1=OP.add)
            # v = u - s
            nc.vector.tensor_sub(v[:], v[:], st_)

            if j == TC - 1:
                nc.gpsimd.dma_start(
                    s_ext[:, k * TC * EL:(k + 1) * TC * EL], sout[k][:])

    nc.finalize()
    return nc


def kernel(x: np.ndarray) -> np.ndarray:
    global LAST_RESULT
    x = np.ascontiguousarray(x, dtype=np.float32)
    assert x.shape == (B, T, D)

    # shard: core c owns d in [c*DLOC, (c+1)*DLOC); element (b, dh, dl):
    # eh = b*4 + dh, el = dl  with d = c*128 + dh*32 + dl
    xs = (x.reshape(B, T, NCORES, EH // B, EL)
           .transpose(2, 0, 3, 1, 4)
           .reshape(NCORES, EH, T * EL))

    nc = _build()
    in_maps = [{"x": np.ascontiguousarray(xs[c])} for c in range(NCORES)]
    LAST_RESULT = run_bass_kernel_spmd(
        nc, in_maps, list(range(NCORES)),
        trace=bool(os.environ.get("ADLIF_TRACE")),
    )
    outs = np.stack([LAST_RESULT.results[c]["out"] for c in range(NCORES)])

    s = (outs.reshape(NCORES, B, EH // B, T, EL)
             .transpose(1, 3, 0, 2, 4)
             .reshape(B, T, D))
    return np.ascontiguousarray(s, dtype=np.float32)


# revision 18
# speedup vs baseline: 2.3278x; 2.3278x over previous
"""AdLIF neuron Bass kernel for 8 Trainium2 NeuronCores.

Recurrence per (b, d) element over T timesteps:
    u  = ALPHA_MEM * v + x_t
    s  = (u >= 2 + 0.1 * a)        computed as (0.1*a - u) <= -2  (bit-exact vs ref)
    a' = ALPHA_ADP * a + s
    v' = u - s

Sharding: D (1024) split across 8 cores -> 128 d's per core.
Per core the 32*128 = 4096 (b,d) elements are laid out as
[eh=128 partitions, el=32 free] and time runs in the free dim of a
[128, T*32] SBUF buffer, so each timestep is one [128, 32] slice.
Host pre-reshapes x to [core, eh, t, el] so the per-core DMA is one
fully contiguous 64KB-per-partition stream.
"""

import os
import numpy as np
from contextlib import ExitStack

import concourse.bass as bass
import concourse.tile as tile
from concourse import bacc, mybir
from concourse.bass_utils import run_bass_kernel_spmd

B, T, D = 32, 512, 1024
NCORES = 8
DLOC = D // NCORES          # 128 d's per core
EH, EL = 128, 32            # 4096 elements per core = EH partitions x EL free
# Uneven t-chunks: small first chunk so compute starts after a ~1MB DMA,
# small last chunk so the final output DMA tail is short.
CHUNKS = [32, 224, 224, 32]
NCHUNK = len(CHUNKS)
CSTART = [sum(CHUNKS[:i]) for i in range(NCHUNK)]

PAD = 16                    # trailing dummy cols on tight-pair producers

ALPHA_MEM = float(np.exp(-1.0 / 20.0))
ALPHA_ADP = float(np.exp(-1.0 / 200.0))

LAST_RESULT = None  # BassKernelResults of the most recent run (for test.py)

F32 = mybir.dt.float32
OP = mybir.AluOpType


def _build():
    nc = bacc.Bacc("TRN2", target_bir_lowering=False, debug=False)
    x_ext = nc.declare_dram_parameter("x", [EH, T * EL], F32, isOutput=False)
    s_ext = nc.declare_dram_parameter("out", [EH, T * EL], F32, isOutput=True)

    with tile.TileContext(nc) as tc, ExitStack() as ctx:
        data = ctx.enter_context(tc.tile_pool(name="data", bufs=1))
        xin = [data.tile([EH, CHUNKS[k] * EL + PAD], F32, name=f"xin{k}", tag=f"x{k}")
               for k in range(NCHUNK)]
        sout = [data.tile([EH, CHUNKS[k] * EL + PAD], F32, name=f"sout{k}", tag=f"s{k}")
                for k in range(NCHUNK)]

        st = ctx.enter_context(tc.tile_pool(name="state", bufs=1))
        # v and w carry PAD trailing scratch columns: the producers of the
        # two tight (adjacent-instruction) dependencies stream PAD extra
        # dummy columns so their real writebacks retire before the next
        # instruction's reads reach them -- replacing semaphore waits.
        v = st.tile([EH, EL + PAD], F32, tag="v")
        a = st.tile([EH, EL], F32, tag="a")
        w = st.tile([EH, EL + PAD], F32, tag="w")
        # (chunk-boundary U-ops carry the DMA wait directly: with all
        # same-engine DVE waits stripped, the single STT sync-wait slot
        # is free for it)

        for k in range(NCHUNK):
            nc.gpsimd.dma_start(
                xin[k][:, 0:CHUNKS[k] * EL],
                x_ext[:, CSTART[k] * EL:(CSTART[k] + CHUNKS[k]) * EL])
            nc.vector.memset(xin[k][:, CHUNKS[k] * EL:], 0.0)

        nc.vector.memset(v[:], 0.0)
        nc.vector.memset(a[:], 0.0)
        nc.vector.memset(w[:], 2.0)   # W = 0.1*a + 2 with a=0 (pad cols stay 2.0)

        # Order [U, CMP, A, V, W] leaves only two distance-1 (tight)
        # same-engine dependencies per step: CMP<-U and A<-CMP. All other
        # deps are >=2 instructions back, where the DVE pipeline overlap
        # can no longer race (empirically validated), so their semaphore
        # waits are stripped below.
        for t in range(T):
            k = next(i for i in range(NCHUNK)
                     if CSTART[i] <= t < CSTART[i] + CHUNKS[i])
            j = t - CSTART[k]
            xt_pad = xin[k][:, j * EL:(j + 1) * EL + PAD]
            st_ = sout[k][:, j * EL:(j + 1) * EL]
            st_pad = sout[k][:, j * EL:(j + 1) * EL + PAD]
            # u = alpha_mem * v + x_t   (u lives in v; streams PAD dummy
            # trailing cols so CMP can follow immediately without a wait)
            nc.vector.scalar_tensor_tensor(v[:], v[:], ALPHA_MEM, xt_pad,
                                           op0=OP.mult, op1=OP.add)
            # s = (u >= W) -> output buffer; PAD spill cols land in the next
            # timestep's slot and are overwritten by its own CMP
            nc.vector.tensor_tensor(st_pad, v[:], w[:], op=OP.is_ge)
            # a = alpha_adp * a + s
            nc.vector.scalar_tensor_tensor(a[:], a[:], ALPHA_ADP, st_,
                                           op0=OP.mult, op1=OP.add)
            # v = u - s
            nc.vector.tensor_sub(v[:, 0:EL], v[:, 0:EL], st_)
            # W = 0.1 * a + 2   (threshold for the next step)
            nc.vector.tensor_scalar(w[:, 0:EL], a[:], 0.1, 2.0,
                                    op0=OP.mult, op1=OP.add)

            if j == CHUNKS[k] - 1:
                nc.gpsimd.dma_start(
                    s_ext[:, CSTART[k] * EL:(CSTART[k] + CHUNKS[k]) * EL],
                    sout[k][:, 0:CHUNKS[k] * EL])

    # The DVE overlaps at most the next instruction with the current one,
    # so a RAW hazard only exists between ADJACENT DVE instructions
    # (distance 1). Tile conservatively emits semaphore waits for longer
    # distances too; each costs ~180ns of event-propagation latency.
    # Strip DVE-on-DVE waits whose producer is >= 2 instructions back,
    # keeping distance-1 waits and all cross-engine waits/updates.
    dve_ordinal = 0
    for f in nc.m.functions:
        for bb in f.blocks:
            for inst in bb.instructions:
                if inst.engine != mybir.EngineType.DVE:
                    continue
                si = inst.sync_info
                has_dve_update = si is not None and any(
                    str(u.ant_name).startswith("DVE") for u in (si.on_update or []))
                if si is not None and si.on_wait:
                    kept = [w for w in si.on_wait
                            if not str(w.ant_name).startswith("DVE")]
                    if len(kept) != len(si.on_wait):
                        si.on_wait = kept
                if has_dve_update:
                    dve_ordinal += 1

    nc.finalize()
    return nc


def kernel(x: np.ndarray) -> np.ndarray:
    global LAST_RESULT
    x = np.ascontiguousarray(x, dtype=np.float32)
    assert x.shape == (B, T, D)

    # shard: core c owns d in [c*DLOC, (c+1)*DLOC); element (b, dh, dl):
    # eh = b*4 + dh, el = dl  with d = c*128 + dh*32 + dl
    xs = (x.reshape(B, T, NCORES, EH // B, EL)
           .transpose(2, 0, 3, 1, 4)
           .reshape(NCORES, EH, T * EL))

    nc = _build()
    in_maps = [{"x": np.ascontiguousarray(xs[c])} for c in range(NCORES)]
    LAST_RESULT = run_bass_kernel_spmd(
        nc, in_maps, list(range(NCORES)),
        trace=bool(os.environ.get("ADLIF_TRACE")),
    )
    outs = np.stack([LAST_RESULT.results[c]["out"] for c in range(NCORES)])

    s = (outs.reshape(NCORES, B, EH // B, T, EL)
             .transpose(1, 3, 0, 2, 4)
             .reshape(B, T, D))
    return np.ascontiguousarray(s, dtype=np.float32)


# revision 20
# speedup vs baseline: 2.3299x; 1.0009x over previous
"""AdLIF neuron Bass kernel for 8 Trainium2 NeuronCores.

Recurrence per (b, d) element over T timesteps (bit-exact vs the jax
reference, including fp rounding order):
    u  = ALPHA_MEM * v + x_t
    s  = (u >= W)                  with threshold state W = fl(fl(0.1*a) + 2)
    a' = ALPHA_ADP * a + s
    v' = u - s
    W' = 0.1 * a' + 2

Sharding: D (1024) split across 8 cores -> 128 d's per core.
Per core the 32*128 = 4096 (b,d) elements are laid out as
[eh=128 partitions, el=32 free] and time runs in the free dim of a
[128, T*32] SBUF buffer, so each timestep is one [128, 32] slice.
Host pre-reshapes x to [core, eh, t, el] so the per-core DMA is one
fully contiguous 64KB-per-partition stream.
"""

import os
import numpy as np
from contextlib import ExitStack

import concourse.bass as bass
import concourse.tile as tile
from concourse import bacc, mybir
from concourse.bass_utils import run_bass_kernel_spmd

B, T, D = 32, 512, 1024
NCORES = 8
DLOC = D // NCORES          # 128 d's per core
EH, EL = 128, 32            # 4096 elements per core = EH partitions x EL free
# Uneven t-chunks: small first chunk so compute starts after a ~1MB DMA,
# small last chunk so the final output DMA tail is short.
CHUNKS = [32, 224, 224, 32]
NCHUNK = len(CHUNKS)
CSTART = [sum(CHUNKS[:i]) for i in range(NCHUNK)]

PAD = 16                    # trailing dummy cols on tight-pair producers

ALPHA_MEM = float(np.exp(-1.0 / 20.0))
ALPHA_ADP = float(np.exp(-1.0 / 200.0))

LAST_RESULT = None  # BassKernelResults of the most recent run (for test.py)

F32 = mybir.dt.float32
OP = mybir.AluOpType


def _build():
    nc = bacc.Bacc("TRN2", target_bir_lowering=False, debug=False)
    x_ext = nc.declare_dram_parameter("x", [EH, T * EL], F32, isOutput=False)
    s_ext = nc.declare_dram_parameter("out", [EH, T * EL], F32, isOutput=True)

    with tile.TileContext(nc) as tc, ExitStack() as ctx:
        data = ctx.enter_context(tc.tile_pool(name="data", bufs=1))
        xin = [data.tile([EH, CHUNKS[k] * EL + PAD], F32, name=f"xin{k}", tag=f"x{k}")
               for k in range(NCHUNK)]
        sout = [data.tile([EH, CHUNKS[k] * EL + PAD], F32, name=f"sout{k}", tag=f"s{k}")
                for k in range(NCHUNK)]

        st = ctx.enter_context(tc.tile_pool(name="state", bufs=1))
        # v and w carry PAD trailing scratch columns: the producers of the
        # two tight (adjacent-instruction) dependencies stream PAD extra
        # dummy columns so their real writebacks retire before the next
        # instruction's reads reach them -- replacing semaphore waits.
        v = st.tile([EH, EL + PAD], F32, tag="v")
        a = st.tile([EH, EL], F32, tag="a")
        w = st.tile([EH, EL + PAD], F32, tag="w")
        # (chunk-boundary U-ops carry the DMA wait directly: with all
        # same-engine DVE waits stripped, the single STT sync-wait slot
        # is free for it)

        for k in range(NCHUNK):
            nc.gpsimd.dma_start(
                xin[k][:, 0:CHUNKS[k] * EL],
                x_ext[:, CSTART[k] * EL:(CSTART[k] + CHUNKS[k]) * EL])
            nc.vector.memset(xin[k][:, CHUNKS[k] * EL:], 0.0)

        nc.vector.memset(v[:], 0.0)
        nc.vector.memset(a[:], 0.0)
        nc.vector.memset(w[:], 2.0)   # W = 0.1*a + 2 with a=0 (pad cols stay 2.0)

        # Order [U, CMP, A, V, W] leaves only two distance-1 (tight)
        # same-engine dependencies per step: CMP<-U and A<-CMP. All other
        # deps are >=2 instructions back, where the DVE pipeline overlap
        # can no longer race (empirically validated), so their semaphore
        # waits are stripped below.
        for t in range(T):
            k = next(i for i in range(NCHUNK)
                     if CSTART[i] <= t < CSTART[i] + CHUNKS[i])
            j = t - CSTART[k]
            xt_pad = xin[k][:, j * EL:(j + 1) * EL + PAD]
            st_ = sout[k][:, j * EL:(j + 1) * EL]
            st_pad = sout[k][:, j * EL:(j + 1) * EL + PAD]
            # u = alpha_mem * v + x_t   (u lives in v; streams PAD dummy
            # trailing cols so CMP can follow immediately without a wait)
            nc.vector.scalar_tensor_tensor(v[:], v[:], ALPHA_MEM, xt_pad,
                                           op0=OP.mult, op1=OP.add)
            # s = (u >= W) -> output buffer; PAD spill cols land in the next
            # timestep's slot and are overwritten by its own CMP
            nc.vector.tensor_tensor(st_pad, v[:], w[:], op=OP.is_ge)
            # a = alpha_adp * a + s
            nc.vector.scalar_tensor_tensor(a[:], a[:], ALPHA_ADP, st_,
                                           op0=OP.mult, op1=OP.add)
            # v = u - s
            nc.vector.tensor_sub(v[:, 0:EL], v[:, 0:EL], st_)
            # W = 0.1 * a + 2   (threshold for the next step)
            nc.vector.tensor_scalar(w[:, 0:EL], a[:], 0.1, 2.0,
                                    op0=OP.mult, op1=OP.add)

            if j == CHUNKS[k] - 1:
                nc.gpsimd.dma_start(
                    s_ext[:, CSTART[k] * EL:(CSTART[k] + CHUNKS[k]) * EL],
                    sout[k][:, 0:CHUNKS[k] * EL])

    _strip_dve_sem_overhead(nc)
    nc.finalize()
    return nc


def _strip_dve_sem_overhead(nc):
    # The DVE overlaps at most the next instruction with the current one,
    # so a RAW hazard only exists between ADJACENT DVE instructions, and the
    # PAD trailing columns on the producers of the two adjacent-dependency
    # pairs (U->CMP, CMP->A) delay the consumer's reads past the producer's
    # writebacks. That makes every Tile-emitted DVE-on-DVE semaphore wait
    # (~180ns event-propagation latency each) redundant -- strip them all.
    # Cross-engine waits (DMA<->DVE) and all semaphore updates are kept.
    for f in nc.m.functions:
        for bb in f.blocks:
            for inst in bb.instructions:
                if inst.engine != mybir.EngineType.DVE:
                    continue
                si = inst.sync_info
                if si is not None and si.on_wait:
                    kept = [w for w in si.on_wait
                            if not str(w.ant_name).startswith("DVE")]
                    if len(kept) != len(si.on_wait):
                        si.on_wait = kept

    # Of the ~2563 DVE semaphore updates only a handful of cumulative
    # threshold values are ever awaited (output DMAs, kernel-tail drain,
    # barrier event-semaphores). Drop the updates nobody waits for and
    # remap the awaited thresholds to the compressed count, removing the
    # per-instruction semaphore-update overhead from the hot loop.
    insts = [i for f in nc.m.functions for bb in f.blocks for i in bb.instructions]

    def dve_sem_names(entries):
        return {str(e.ant_name) for e in entries if str(e.ant_name).startswith("DVE")}

    sems = set()
    for i in insts:
        if i.sync_info:
            sems |= dve_sem_names(i.sync_info.on_update or [])
    for sem in sems:
        awaited = set()
        for i in insts:
            si = i.sync_info
            if si is None:
                continue
            for wt in (si.on_wait or []):
                if str(wt.ant_name) == sem:
                    awaited.add(wt.wait_value)
        ordinal = 0
        remap = {}
        kept_count = 0
        for i in insts:
            si = i.sync_info
            if si is None:
                continue
            ups = [u for u in (si.on_update or []) if str(u.ant_name) == sem]
            if not ups:
                continue
            ordinal += 1
            if ordinal in awaited:
                kept_count += 1
                remap[ordinal] = kept_count
            else:
                si.on_update = [u for u in si.on_update
                                if str(u.ant_name) != sem]
        for i in insts:
            si = i.sync_info
            if si is None:
                continue
            for wt in (si.on_wait or []):
                if str(wt.ant_name) == sem:
                    wt.wait_value = remap[wt.wait_value]


def kernel(x: np.ndarray) -> np.ndarray:
    global LAST_RESULT
    x = np.ascontiguousarray(x, dtype=np.float32)
    assert x.shape == (B, T, D)

    # shard: core c owns d in [c*DLOC, (c+1)*DLOC); element (b, dh, dl):
    # eh = b*4 + dh, el = dl  with d = c*128 + dh*32 + dl
    xs = (x.reshape(B, T, NCORES, EH // B, EL)
           .transpose(2, 0, 3, 1, 4)
           .reshape(NCORES, EH, T * EL))

    nc = _build()
    in_maps = [{"x": np.ascontiguousarray(xs[c])} for c in range(NCORES)]
    LAST_RESULT = run_bass_kernel_spmd(
        nc, in_maps, list(range(NCORES)),
        trace=bool(os.environ.get("ADLIF_TRACE")),
    )
    outs = np.stack([LAST_RESULT.results[c]["out"] for c in range(NCORES)])

    s = (outs.reshape(NCORES, B, EH // B, T, EL)
             .transpose(1, 3, 0, 2, 4)
             .reshape(B, T, D))
    return np.ascontiguousarray(s, dtype=np.float32)
